# revision 78
# baseline (speedup 1.0000x reference)
"""Causal MHA with relative bias + attention-mean output, on 8 trn2 cores.

Reference computation (per batch b):
  q/k/v = x @ w* + b*           -> [S, D] split into H=8 heads of dk=64
  scores = q k^T / 8 + rel_bias[:, :S, :S], causal masked
  attn = softmax(scores); ctx = attn @ v; out = ctx @ wo + bo
  second output: attn.mean(over heads)  -> [B, S, S]

Sharding: data-parallel over batch, 4 batches per core, weights/bias/rel_bias
replicated. One SPMD NEFF runs on cores 0-7.

Per-core dataflow (S=480 = 4 tiles of 120, D=512 = 4 k-tiles of 128):
  - x[b] -> PE-transpose -> xT [d, s] (f32r)
  - Q^T = wq^T @ xT (heads on partitions, scaled by 1/8), K^T likewise
  - V natural [s, d] with a ones column appended per head (row 64 of the PV
    matmul output then carries the softmax denominators for free)
  - scores are built TRANSPOSED (st[k, q]): rel_bias+causal mask (transposed
    once at setup, bf16) is injected into PSUM with an identity matmul, then
    the K^T/Q^T matmul accumulates on top; exp on the scalar engine
  - ctx^T = matmul(V_aug, E); per-q 1/rowsum (replicated across partitions
    with a K=1 ones matmul) folded into the PSUM->SBUF copy;
    out = matmul(ctx^T, wo) + bo
  - attn-mean accumulated unnormalized-by-1/H in [k, q] layout (E * rr_rep),
    PE-transposed back to natural [q, k] at the end; the 1/8 rides the final
    PSUM->SBUF copy.
"""

import numpy as np

import concourse.bass as bass
import concourse.mybir as mybir
import concourse.tile as tile
from concourse import bacc
from concourse.bass_utils import run_bass_kernel_spmd

F32 = mybir.dt.float32
F32R = mybir.dt.float32r
BF16 = mybir.dt.bfloat16
AF = mybir.ActivationFunctionType
ALU = mybir.AluOpType

B = 32
S = 480
D = 512
H = 8
DK = 64
MAX_LEN = 500
N_CORES = 8
BPC = B // N_CORES  # batches per core
ST = 4  # s tiles of 120
SP = S // ST  # 120
KO = D // 128  # 4
MASK_VAL = -1e9


def build():
    nc = bacc.Bacc("TRN2", num_devices=N_CORES)

    x = nc.dram_tensor("x", [BPC, S, D], F32, kind="ExternalInput")
    wq = nc.dram_tensor("wq", [D, D], F32, kind="ExternalInput")
    wk = nc.dram_tensor("wk", [D, D], F32, kind="ExternalInput")
    wv = nc.dram_tensor("wv", [D, D], F32, kind="ExternalInput")
    wo = nc.dram_tensor("wo", [D, D], F32, kind="ExternalInput")
    bq = nc.dram_tensor("bq", [D], F32, kind="ExternalInput")
    bk = nc.dram_tensor("bk", [D], F32, kind="ExternalInput")
    bv = nc.dram_tensor("bv", [D], F32, kind="ExternalInput")
    bo = nc.dram_tensor("bo", [D], F32, kind="ExternalInput")
    rel_bias = nc.dram_tensor("rel_bias", [H, MAX_LEN, MAX_LEN], F32, kind="ExternalInput")

    out = nc.dram_tensor("out", [BPC, S, D], F32, kind="ExternalOutput")
    amean = nc.dram_tensor("amean", [BPC, S, S], F32, kind="ExternalOutput")

    with nc.allow_low_precision("f32r pipeline is deliberate"), tile.TileContext(nc) as tc:
        with (
            tc.tile_pool(name="const", bufs=1) as const,
            tc.tile_pool(name="stage", bufs=2) as stage,
            tc.tile_pool(name="xstage", bufs=1) as xstage,
            tc.tile_pool(name="p2", bufs=4) as p2,
            tc.tile_pool(name="tpool", bufs=10) as tpool,
            tc.tile_pool(name="perb", bufs=2) as perb,
            tc.tile_pool(name="single", bufs=1) as single,
            tc.tile_pool(name="epool", bufs=4) as epool,
            tc.tile_pool(name="opool", bufs=2) as opool,
            tc.tile_pool(name="ps_big", bufs=2, space="PSUM") as ps_big,
            tc.tile_pool(name="ps_st", bufs=3, space="PSUM") as ps_st,
            tc.tile_pool(name="ps_ctx", bufs=2, space="PSUM") as ps_ctx,
            tc.tile_pool(name="ps_small", bufs=1, space="PSUM") as ps_small,
        ):
            # ---------------- setup: identities / ones ----------------
            ident = const.tile([128, 128], F32)
            nc.gpsimd.memset(ident[:], 0.0)
            nc.gpsimd.affine_select(
                out=ident[:], in_=ident[:], compare_op=ALU.not_equal, fill=1.0,
                base=0, pattern=[[-1, 128]], channel_multiplier=1,
            )
            ident_bf = const.tile([SP, SP], BF16)
            nc.gpsimd.memset(ident_bf[:], 0.0)
            nc.gpsimd.affine_select(
                out=ident_bf[:], in_=ident_bf[:], compare_op=ALU.not_equal, fill=1.0,
                base=0, pattern=[[-1, SP]], channel_multiplier=1,
            )
            ones_f = const.tile([1, 128], F32, tag="ones_f")
            nc.vector.memset(ones_f[:], 1.0)
            ones_r = const.tile([1, 128], F32R, tag="ones_r")
            nc.vector.tensor_copy(ones_r[:], ones_f[:])
            ones_col = const.tile([SP, 1], F32, tag="ones_col")
            nc.vector.memset(ones_col[:], 1.0)

            # ---------------- setup: weights -> f32r ----------------
            # [128, ko, 512]: partition = d_in % 128, ko = d_in // 128
            w_r = {}
            for i, (name, w) in enumerate(
                (("wq", wq), ("wk", wk), ("wv", wv), ("wo", wo))
            ):
                st_t = stage.tile([128, KO, D], F32, tag="stage")
                dma_eng = (nc.scalar, nc.gpsimd, nc.scalar, nc.gpsimd)[i]
                dma_eng.dma_start(st_t[:], w.rearrange("(ko p) n -> p ko n", p=128))
                wr = const.tile([128, KO, D], F32R, tag=f"w_{name}")
                nc.vector.tensor_copy(wr[:], st_t[:])
                w_r[name] = wr

            # ---------------- setup: bias vectors ----------------
            # partition layout [128, ko] for per-partition ACT bias (Q^T/K^T)
            bq_p = const.tile([128, KO], F32, tag="bq_p")
            bk_p = const.tile([128, KO], F32, tag="bk_p")
            nc.sync.dma_start(bq_p[:], bq.rearrange("(ko p) -> p ko", p=128))
            nc.sync.dma_start(bk_p[:], bk.rearrange("(ko p) -> p ko", p=128))
            bq8_p = const.tile([128, KO], F32, tag="bq8_p")
            nc.vector.tensor_scalar_mul(bq8_p[:], bq_p[:], 0.125)
            # bv/bo replicated across partitions via K=1 ones matmul
            bv_f = const.tile([1, D], F32, tag="bv_f")
            bo_f = const.tile([1, D], F32, tag="bo_f")
            nc.sync.dma_start(bv_f[:], bv[None, :])
            nc.sync.dma_start(bo_f[:], bo[None, :])
            bv_rep = const.tile([SP, H, DK], F32, tag="bv_rep")
            bo_rep = const.tile([SP, D], F32, tag="bo_rep")
            for vec, rep in ((bv_f, bv_rep), (bo_f, bo_rep)):
                prep = ps_big.tile([128, 512], F32, tag="ps_big")
                nc.tensor.matmul(
                    prep[:SP, :], ones_f[:, :SP], vec[:], start=True, stop=True
                )
                if rep is bv_rep:
                    nc.vector.tensor_copy(
                        rep[:], prep[:SP, :].rearrange("p (h d) -> p h d", h=H)
                    )
                else:
                    nc.vector.tensor_copy(rep[:], prep[:SP, :])

            # ---------------- transposed masked rel_bias (lazy, emitted
            # just-in-time inside batch 0 so it interleaves with compute) ----
            bT = {}

            def setup_bias_head(h):
                nat_full = stage.tile([128, KO, D], F32, tag="stage", name=f"nat_{h}")
                nat = nat_full[:SP, :, :S]
                nc.scalar.dma_start(
                    nat[:], rel_bias[h, :S, :S].rearrange("(qt p) k -> p qt k", p=SP)
                )
                # causal mask: only the diagonal blocks are partially
                # masked; blocks with k-tile > q-tile are never read (the
                # bias matmuls are sliced to q >= kt*SP)
                for qt in range(ST):
                    nc.gpsimd.affine_select(
                        out=nat[:, qt, qt * SP:(qt + 1) * SP],
                        in_=nat[:, qt, qt * SP:(qt + 1) * SP],
                        compare_op=ALU.is_ge, fill=MASK_VAL,
                        base=0, pattern=[[-1, SP]], channel_multiplier=1,
                    )
                bt_h = const.tile([SP, ST, S], BF16, tag=f"bT_{h}", name=f"bT_{h}")
                for kt in range(ST):
                    ptr = ps_big.tile([128, 512], F32, tag="ps_big", name="ptr_b")
                    for qt in range(kt, ST):
                        nc.tensor.transpose(
                            ptr[:SP, qt * SP:(qt + 1) * SP],
                            nat[:, qt, kt * SP:(kt + 1) * SP],
                            ident[:SP, :SP],
                        )
                    if kt % 2 == 0:
                        nc.vector.tensor_copy(
                            bt_h[:, kt, kt * SP:], ptr[:SP, kt * SP:S]
                        )
                    else:
                        nc.scalar.copy(
                            bt_h[:, kt, kt * SP:], ptr[:SP, kt * SP:S]
                        )
                bT[h] = bt_h

            # ---------------- per batch (software-pipelined) ----------------
            # emission order: proj(b) -> tail(b-1) -> attention(b); the tail
            # of a batch (out-proj + mean transposes) overlaps the next
            # batch's projection phase on otherwise-idle engines.
            state = {}

            def proj_phase(b):
                xs_full = xstage.tile([128, KO, D], F32, tag="xstage",
                                      name=f"xs_{b}")
                xs = xs_full[:SP]
                nc.sync.dma_start(
                    xs[:], x[b].rearrange("(st p) d -> p st d", p=SP)
                )

                # xT [128, ko, 480] f32r via PE transposes
                xT = single.tile([128, KO, S], F32R, tag="xT", name=f"xT_{b}")
                for ko in range(KO):
                    ptr = ps_big.tile([128, 512], F32, tag="ps_big", name="ptr_x")
                    for st_i in range(ST):
                        nc.tensor.transpose(
                            ptr[:, st_i * SP:(st_i + 1) * SP],
                            xs[:, st_i, ko * 128:(ko + 1) * 128],
                            ident[:SP, :SP],
                        )
                    nc.vector.tensor_copy(xT[:, ko, :], ptr[:, :S])

                # Q^T (scaled 1/8) and K^T: [128, ko, 480] f32r
                QT = perb.tile([128, KO, S], F32R, tag="QT", name=f"QT_{b}")
                KT = perb.tile([128, KO, S], F32R, tag="KT", name=f"KT_{b}")
                for m in range(KO):
                    pq = ps_big.tile([128, 512], F32, tag="ps_big", name="pq")
                    for ko in range(KO):
                        nc.tensor.matmul(
                            pq[:, :S],
                            w_r["wq"][:, ko, m * 128:(m + 1) * 128],
                            xT[:, ko, :],
                            start=(ko == 0), stop=(ko == KO - 1),
                        )
                    nc.scalar.activation(
                        QT[:, m, :], pq[:, :S], AF.Identity,
                        bias=bq8_p[:, m:m + 1], scale=0.125,
                    )
                    pk = ps_big.tile([128, 512], F32, tag="ps_big", name="pk")
                    for ko in range(KO):
                        nc.tensor.matmul(
                            pk[:, :S],
                            w_r["wk"][:, ko, m * 128:(m + 1) * 128],
                            xT[:, ko, :],
                            start=(ko == 0), stop=(ko == KO - 1),
                        )
                    nc.scalar.activation(
                        KT[:, m, :], pk[:, :S], AF.Identity,
                        bias=bk_p[:, m:m + 1], scale=1.0,
                    )

                # V natural with ones column: [120, st, H, 65] bf16
                V = perb.tile([SP, ST, H, DK + 1], BF16, tag="V", name=f"V_{b}")
                for st_i in range(ST):
                    pv = ps_big.tile([128, 512], F32, tag="ps_big", name="pv")
                    for ko in range(KO):
                        nc.tensor.matmul(
                            pv[:SP, :],
                            xT[:, ko, st_i * SP:(st_i + 1) * SP],
                            w_r["wv"][:, ko, :],
                            start=(ko == 0), stop=(ko == KO - 1),
                        )
                    nc.vector.tensor_tensor(
                        V[:, st_i, :, :DK],
                        pv[:SP, :].rearrange("p (h d) -> p h d", h=H),
                        bv_rep[:],
                        ALU.add,
                    )
                    nc.vector.tensor_copy(
                        V[:, st_i, :, DK],
                        ones_col[:, :].to_broadcast((SP, H)),
                    )
                state[b] = {"QT": QT, "KT": KT, "V": V}

            def attn_phase(b):
                st_b = state[b]
                QT, KT, V = st_b["QT"], st_b["KT"], st_b["V"]
                CT = single.tile([128, KO, S], F32R, tag="CT", name=f"CT_{b}")
                mean_acc = single.tile([SP, ST, S], F32, tag="mean_acc",
                                       name=f"macc_{b}")
                st_b["CT"] = CT
                st_b["mean_acc"] = mean_acc
                for hp in range(H // 2):
                    pair = (2 * hp, 2 * hp + 1)
                    if b == 0:
                        # prefetch the NEXT pair's bias while this one computes
                        for h in (2 * hp + 2, 2 * hp + 3):
                            if h < H and h not in bT:
                                setup_bias_head(h)
                    Es = {}
                    for h in pair:
                        Es[h] = epool.tile([SP, ST, S], BF16, tag="E", name=f"E_{h}")
                    pcs = {}
                    for h in pair:
                        pcs[h] = ps_ctx.tile(
                            [DK + 1, S], F32, tag="ps_ctx", name=f"pc_{h}"
                        )
                    for kt in range(ST):
                        # columns q < kt*SP are fully causal-masked: skip them
                        off = kt * SP
                        # f32r matmuls below 256 rows fall to the 4x-slow
                        # path, so only slice the score matmul when the
                        # remaining width stays >= 256
                        soff = off if S - off >= 256 else 0
                        psts = {}
                        for h in pair:
                            pst = ps_st.tile([SP, S], F32, tag="ps_st",
                                             name=f"pst_{h}")
                            psts[h] = pst
                            nc.tensor.matmul(
                                pst[:, off:], ident_bf[:], bT[h][:, kt, off:],
                                start=True, stop=False,
                            )
                        for h in pair:
                            ko_h, base = h // 2, (h % 2) * DK
                            nc.tensor.matmul(
                                psts[h][:, soff:],
                                KT[base:base + DK, ko_h, kt * SP:(kt + 1) * SP],
                                QT[base:base + DK, ko_h, soff:],
                                start=False, stop=True,
                            )
                        for h in pair:
                            nc.scalar.activation(
                                Es[h][:, kt, off:], psts[h][:, off:], AF.Exp
                            )
                        # PV accumulation interleaved: starts after first exp
                        for h in pair:
                            nc.tensor.matmul(
                                pcs[h][:, off:],
                                V[:, kt, h, :],
                                Es[h][:, kt, off:],
                                start=(kt == 0), stop=(kt == ST - 1),
                            )

                    Ts = {pair[0]: {}, pair[1]: {}}
                    for h in pair:
                        ko_h, base = h // 2, (h % 2) * DK
                        E = Es[h]
                        pc = pcs[h]
                        # rr = 1/rowsum, replicated across partitions (K=1 mm)
                        rr = p2.tile([1, S], F32R, tag="rr", name=f"rr_{h}")
                        nc.vector.reciprocal(rr[:], pc[DK:DK + 1, :])
                        prr = ps_small.tile([SP, S], F32, tag="ps_small", name="prr")
                        nc.tensor.matmul(
                            prr[:], ones_r[:, :SP], rr[:], start=True, stop=True
                        )
                        rr_rep = p2.tile([SP, S], F32, tag="rr_rep")
                        nc.scalar.copy(rr_rep[:], prr[:])
                        rr_bf = p2.tile([SP, S], BF16, tag="rr_rep_bf")
                        nc.scalar.copy(rr_bf[:], prr[:])
                        nc.vector.tensor_tensor(
                            CT[base:base + DK, ko_h, :],
                            pc[:DK, :],
                            rr_rep[:DK, :],
                            ALU.mult,
                        )
                        for kt in range(ST):
                            off = kt * SP
                            t = tpool.tile([SP, S], BF16, tag="mtmp",
                                           name=f"T_{h}_{kt}")
                            Ts[h][kt] = t
                            nc.vector.tensor_tensor(
                                t[:, off:], E[:, kt, off:], rr_bf[:, off:],
                                ALU.mult,
                            )
                    # fold the pair into mean_acc
                    for kt in range(ST):
                        off = kt * SP
                        ta, tb = Ts[pair[0]][kt], Ts[pair[1]][kt]
                        if hp == 0:
                            eng = nc.vector if kt % 2 == 0 else nc.gpsimd
                            eng.tensor_tensor(
                                mean_acc[:, kt, off:], ta[:, off:], tb[:, off:],
                                ALU.add,
                            )
                        else:
                            psum_t = p2.tile([SP, S], BF16, tag="psum_t",
                                             name=f"ps_{hp}_{kt}")
                            nc.vector.tensor_tensor(
                                psum_t[:, off:], ta[:, off:], tb[:, off:], ALU.add
                            )
                            eng = nc.vector if kt % 2 == 0 else nc.gpsimd
                            eng.tensor_tensor(
                                mean_acc[:, kt, off:], mean_acc[:, kt, off:],
                                psum_t[:, off:], ALU.add,
                            )

            def tail_phase(b):
                st_b = state[b]
                CT, mean_acc = st_b["CT"], st_b["mean_acc"]
                # output projection: out[q, :] = ctx @ wo + bo
                for qt in range(ST):
                    po = ps_big.tile([128, 512], F32, tag="ps_big", name="po")
                    for ko in range(KO):
                        nc.tensor.matmul(
                            po[:SP, :],
                            CT[:, ko, qt * SP:(qt + 1) * SP],
                            w_r["wo"][:, ko, :],
                            start=(ko == 0), stop=(ko == KO - 1),
                        )
                    osb = opool.tile([SP, D], F32, tag="ob", name="osb")
                    nc.vector.tensor_tensor(osb[:], po[:SP, :], bo_rep[:], ALU.add)
                    nc.sync.dma_start(out[b, qt * SP:(qt + 1) * SP, :], osb[:])

                # attn-mean: transpose back to [q, k]; 1/H rides the copy
                for qt in range(ST):
                    # blocks with kt > qt are entirely in the causal-masked
                    # region (k > q): the output there is exactly zero
                    w = (qt + 1) * SP
                    ptr = ps_small.tile([SP, S], F32, tag="ps_small", name="ptr_m")
                    for kt in range(qt + 1):
                        nc.tensor.transpose(
                            ptr[:, kt * SP:(kt + 1) * SP],
                            mean_acc[:, kt, qt * SP:(qt + 1) * SP],
                            ident[:SP, :SP],
                        )
                    msb_full = opool.tile([SP, D], F32, tag="ob", name="msb")
                    msb = msb_full[:, :S]
                    if w < S:
                        nc.gpsimd.memset(msb[:, w:], 0.0)
                    nc.scalar.mul(msb[:, :w], ptr[:, :w], 0.125)
                    nc.sync.dma_start(
                        amean[b, qt * SP:(qt + 1) * SP, :], msb[:]
                    )

            for b in range(BPC):
                proj_phase(b)
                if b == 0:
                    setup_bias_head(0)
                    setup_bias_head(1)
                else:
                    tail_phase(b - 1)
                attn_phase(b)
            tail_phase(BPC - 1)

    nc.finalize()
    return nc


_NC_CACHE = None


def _get_nc():
    global _NC_CACHE
    if _NC_CACHE is None:
        _NC_CACHE = build()
    return _NC_CACHE


def run(inputs: dict, trace: bool = False):
    nc = _get_nc()
    shared = {k: np.ascontiguousarray(np.asarray(v, dtype=np.float32))
              for k, v in inputs.items() if k != "x"}
    xfull = np.ascontiguousarray(np.asarray(inputs["x"], dtype=np.float32))
    in_maps = [
        {"x": xfull[c * BPC:(c + 1) * BPC], **shared} for c in range(N_CORES)
    ]
    res = run_bass_kernel_spmd(
        nc, in_maps, core_ids=list(range(N_CORES)), trace=trace
    )
    out = np.concatenate([r["out"] for r in res.results], axis=0)
    amean = np.concatenate([r["amean"] for r in res.results], axis=0)
    return (out, amean), res


def kernel(**inputs) -> tuple[np.ndarray, np.ndarray]:
    (out, amean), _ = run(inputs, trace=False)
    return out, amean


# revision 81
# speedup vs baseline: 1.0047x; 1.0047x over previous
"""Causal MHA with relative bias + attention-mean output, on 8 trn2 cores.

Reference computation (per batch b):
  q/k/v = x @ w* + b*           -> [S, D] split into H=8 heads of dk=64
  scores = q k^T / 8 + rel_bias[:, :S, :S], causal masked
  attn = softmax(scores); ctx = attn @ v; out = ctx @ wo + bo
  second output: attn.mean(over heads)  -> [B, S, S]

Sharding: data-parallel over batch, 4 batches per core, weights/bias/rel_bias
replicated. One SPMD NEFF runs on cores 0-7.

Per-core dataflow (S=480 = 4 tiles of 120, D=512 = 4 k-tiles of 128):
  - x[b] -> PE-transpose -> xT [d, s] (f32r)
  - Q^T = wq^T @ xT (heads on partitions, scaled by 1/8), K^T likewise
  - V natural [s, d] with a ones column appended per head (row 64 of the PV
    matmul output then carries the softmax denominators for free)
  - scores are built TRANSPOSED (st[k, q]): rel_bias+causal mask (transposed
    once at setup, bf16) is injected into PSUM with an identity matmul, then
    the K^T/Q^T matmul accumulates on top; exp on the scalar engine
  - ctx^T = matmul(V_aug, E); per-q 1/rowsum (replicated across partitions
    with a K=1 ones matmul) folded into the PSUM->SBUF copy;
    out = matmul(ctx^T, wo) + bo
  - attn-mean accumulated unnormalized-by-1/H in [k, q] layout (E * rr_rep),
    PE-transposed back to natural [q, k] at the end; the 1/8 rides the final
    PSUM->SBUF copy.
"""

import numpy as np

import concourse.bass as bass
import concourse.mybir as mybir
import concourse.tile as tile
from concourse import bacc
from concourse.bass_utils import run_bass_kernel_spmd

F32 = mybir.dt.float32
F32R = mybir.dt.float32r
BF16 = mybir.dt.bfloat16
AF = mybir.ActivationFunctionType
ALU = mybir.AluOpType

B = 32
S = 480
D = 512
H = 8
DK = 64
MAX_LEN = 500
N_CORES = 8
BPC = B // N_CORES  # batches per core
ST = 4  # s tiles of 120
SP = S // ST  # 120
KO = D // 128  # 4
MASK_VAL = -1e9


def build():
    nc = bacc.Bacc("TRN2", num_devices=N_CORES)

    x = nc.dram_tensor("x", [BPC, S, D], F32, kind="ExternalInput")
    wq = nc.dram_tensor("wq", [D, D], F32, kind="ExternalInput")
    wk = nc.dram_tensor("wk", [D, D], F32, kind="ExternalInput")
    wv = nc.dram_tensor("wv", [D, D], F32, kind="ExternalInput")
    wo = nc.dram_tensor("wo", [D, D], F32, kind="ExternalInput")
    bq = nc.dram_tensor("bq", [D], F32, kind="ExternalInput")
    bk = nc.dram_tensor("bk", [D], F32, kind="ExternalInput")
    bv = nc.dram_tensor("bv", [D], F32, kind="ExternalInput")
    bo = nc.dram_tensor("bo", [D], F32, kind="ExternalInput")
    rel_bias = nc.dram_tensor("rel_bias", [H, MAX_LEN, MAX_LEN], F32, kind="ExternalInput")

    out = nc.dram_tensor("out", [BPC, S, D], F32, kind="ExternalOutput")
    amean = nc.dram_tensor("amean", [BPC, S, S], F32, kind="ExternalOutput")

    with nc.allow_low_precision("f32r pipeline is deliberate"), tile.TileContext(nc) as tc:
        with (
            tc.tile_pool(name="const", bufs=1) as const,
            tc.tile_pool(name="stage", bufs=2) as stage,
            tc.tile_pool(name="xstage", bufs=1) as xstage,
            tc.tile_pool(name="p2", bufs=4) as p2,
            tc.tile_pool(name="tpool", bufs=10) as tpool,
            tc.tile_pool(name="perb", bufs=2) as perb,
            tc.tile_pool(name="single", bufs=1) as single,
            tc.tile_pool(name="epool", bufs=4) as epool,
            tc.tile_pool(name="opool", bufs=2) as opool,
            tc.tile_pool(name="ps_big", bufs=2, space="PSUM") as ps_big,
            tc.tile_pool(name="ps_st", bufs=3, space="PSUM") as ps_st,
            tc.tile_pool(name="ps_ctx", bufs=2, space="PSUM") as ps_ctx,
            tc.tile_pool(name="ps_small", bufs=1, space="PSUM") as ps_small,
        ):
            # ---------------- setup: identities / ones ----------------
            ident = const.tile([128, 128], F32)
            nc.gpsimd.memset(ident[:], 0.0)
            nc.gpsimd.affine_select(
                out=ident[:], in_=ident[:], compare_op=ALU.not_equal, fill=1.0,
                base=0, pattern=[[-1, 128]], channel_multiplier=1,
            )
            ident_bf = const.tile([SP, SP], BF16)
            nc.gpsimd.memset(ident_bf[:], 0.0)
            nc.gpsimd.affine_select(
                out=ident_bf[:], in_=ident_bf[:], compare_op=ALU.not_equal, fill=1.0,
                base=0, pattern=[[-1, SP]], channel_multiplier=1,
            )
            ones_f = const.tile([1, 128], F32, tag="ones_f")
            nc.vector.memset(ones_f[:], 1.0)
            ones_r = const.tile([1, 128], F32R, tag="ones_r")
            nc.vector.tensor_copy(ones_r[:], ones_f[:])
            ones_col = const.tile([SP, 1], F32, tag="ones_col")
            nc.vector.memset(ones_col[:], 1.0)

            # ---------------- setup: weights -> f32r ----------------
            # [128, ko, 512]: partition = d_in % 128, ko = d_in // 128
            w_r = {}
            for i, (name, w) in enumerate(
                (("wq", wq), ("wk", wk), ("wv", wv), ("wo", wo))
            ):
                st_t = stage.tile([128, KO, D], F32, tag="stage")
                dma_eng = (nc.scalar, nc.gpsimd, nc.scalar, nc.gpsimd)[i]
                dma_eng.dma_start(st_t[:], w.rearrange("(ko p) n -> p ko n", p=128))
                wr = const.tile([128, KO, D], F32R, tag=f"w_{name}")
                nc.vector.tensor_copy(wr[:], st_t[:])
                w_r[name] = wr

            # ---------------- setup: bias vectors ----------------
            # partition layout [128, ko] for per-partition ACT bias (Q^T/K^T)
            bq_p = const.tile([128, KO], F32, tag="bq_p")
            bk_p = const.tile([128, KO], F32, tag="bk_p")
            nc.sync.dma_start(bq_p[:], bq.rearrange("(ko p) -> p ko", p=128))
            nc.sync.dma_start(bk_p[:], bk.rearrange("(ko p) -> p ko", p=128))
            bq8_p = const.tile([128, KO], F32, tag="bq8_p")
            nc.vector.tensor_scalar_mul(bq8_p[:], bq_p[:], 0.125)
            # bv/bo replicated across partitions via K=1 ones matmul
            bv_f = const.tile([1, D], F32, tag="bv_f")
            bo_f = const.tile([1, D], F32, tag="bo_f")
            nc.sync.dma_start(bv_f[:], bv[None, :])
            nc.sync.dma_start(bo_f[:], bo[None, :])
            bv_rep = const.tile([SP, H, DK], F32, tag="bv_rep")
            bo_rep = const.tile([SP, D], F32, tag="bo_rep")
            for vec, rep in ((bv_f, bv_rep), (bo_f, bo_rep)):
                prep = ps_big.tile([128, 512], F32, tag="ps_big")
                nc.tensor.matmul(
                    prep[:SP, :], ones_f[:, :SP], vec[:], start=True, stop=True
                )
                if rep is bv_rep:
                    nc.vector.tensor_copy(
                        rep[:], prep[:SP, :].rearrange("p (h d) -> p h d", h=H)
                    )
                else:
                    nc.vector.tensor_copy(rep[:], prep[:SP, :])

            # ---------------- transposed masked rel_bias (lazy, emitted
            # just-in-time inside batch 0 so it interleaves with compute) ----
            bT = {}

            def setup_bias_head(h):
                nat_full = stage.tile([128, KO, D], F32, tag="stage", name=f"nat_{h}")
                nat = nat_full[:SP, :, :S]
                nc.scalar.dma_start(
                    nat[:], rel_bias[h, :S, :S].rearrange("(qt p) k -> p qt k", p=SP)
                )
                # causal mask: only the diagonal blocks are partially
                # masked; blocks with k-tile > q-tile are never read (the
                # bias matmuls are sliced to q >= kt*SP)
                for qt in range(ST):
                    nc.gpsimd.affine_select(
                        out=nat[:, qt, qt * SP:(qt + 1) * SP],
                        in_=nat[:, qt, qt * SP:(qt + 1) * SP],
                        compare_op=ALU.is_ge, fill=MASK_VAL,
                        base=0, pattern=[[-1, SP]], channel_multiplier=1,
                    )
                bt_h = const.tile([SP, ST, S], BF16, tag=f"bT_{h}", name=f"bT_{h}")
                for kt in range(ST):
                    ptr = ps_big.tile([128, 512], F32, tag="ps_big", name="ptr_b")
                    for qt in range(kt, ST):
                        nc.tensor.transpose(
                            ptr[:SP, qt * SP:(qt + 1) * SP],
                            nat[:, qt, kt * SP:(kt + 1) * SP],
                            ident[:SP, :SP],
                        )
                    if kt % 2 == 0:
                        nc.vector.tensor_copy(
                            bt_h[:, kt, kt * SP:], ptr[:SP, kt * SP:S]
                        )
                    else:
                        nc.scalar.copy(
                            bt_h[:, kt, kt * SP:], ptr[:SP, kt * SP:S]
                        )
                bT[h] = bt_h

            # ---------------- per batch (software-pipelined) ----------------
            # emission order: proj(b) -> tail(b-1) -> attention(b); the tail
            # of a batch (out-proj + mean transposes) overlaps the next
            # batch's projection phase on otherwise-idle engines.
            state = {}

            def proj_phase(b):
                xs_full = xstage.tile([128, KO, D], F32, tag="xstage",
                                      name=f"xs_{b}")
                xs = xs_full[:SP]
                nc.sync.dma_start(
                    xs[:], x[b].rearrange("(st p) d -> p st d", p=SP)
                )

                # xT [128, ko, 480] f32r via PE transposes
                xT = single.tile([128, KO, S], F32R, tag="xT", name=f"xT_{b}")
                for ko in range(KO):
                    ptr = ps_big.tile([128, 512], F32, tag="ps_big", name="ptr_x")
                    for st_i in range(ST):
                        nc.tensor.transpose(
                            ptr[:, st_i * SP:(st_i + 1) * SP],
                            xs[:, st_i, ko * 128:(ko + 1) * 128],
                            ident[:SP, :SP],
                        )
                    nc.vector.tensor_copy(xT[:, ko, :], ptr[:, :S])

                # Q^T (scaled 1/8) and K^T: [128, ko, 480] f32r
                QT = perb.tile([128, KO, S], F32R, tag="QT", name=f"QT_{b}")
                KT = perb.tile([128, KO, S], F32R, tag="KT", name=f"KT_{b}")
                for m in range(KO):
                    pq = ps_big.tile([128, 512], F32, tag="ps_big", name="pq")
                    for ko in range(KO):
                        nc.tensor.matmul(
                            pq[:, :S],
                            w_r["wq"][:, ko, m * 128:(m + 1) * 128],
                            xT[:, ko, :],
                            start=(ko == 0), stop=(ko == KO - 1),
                        )
                    nc.scalar.activation(
                        QT[:, m, :], pq[:, :S], AF.Identity,
                        bias=bq8_p[:, m:m + 1], scale=0.125,
                    )
                    pk = ps_big.tile([128, 512], F32, tag="ps_big", name="pk")
                    for ko in range(KO):
                        nc.tensor.matmul(
                            pk[:, :S],
                            w_r["wk"][:, ko, m * 128:(m + 1) * 128],
                            xT[:, ko, :],
                            start=(ko == 0), stop=(ko == KO - 1),
                        )
                    nc.scalar.activation(
                        KT[:, m, :], pk[:, :S], AF.Identity,
                        bias=bk_p[:, m:m + 1], scale=1.0,
                    )

                # V natural with ones column: [120, st, H, 65] bf16
                V = perb.tile([SP, ST, H, DK + 1], BF16, tag="V", name=f"V_{b}")
                for st_i in range(ST):
                    pv = ps_big.tile([128, 512], F32, tag="ps_big", name="pv")
                    for ko in range(KO):
                        nc.tensor.matmul(
                            pv[:SP, :],
                            xT[:, ko, st_i * SP:(st_i + 1) * SP],
                            w_r["wv"][:, ko, :],
                            start=(ko == 0), stop=(ko == KO - 1),
                        )
                    nc.vector.tensor_tensor(
                        V[:, st_i, :, :DK],
                        pv[:SP, :].rearrange("p (h d) -> p h d", h=H),
                        bv_rep[:],
                        ALU.add,
                    )
                    nc.vector.tensor_copy(
                        V[:, st_i, :, DK],
                        ones_col[:, :].to_broadcast((SP, H)),
                    )
                state[b] = {"QT": QT, "KT": KT, "V": V}

            def attn_phase(b):
                st_b = state[b]
                QT, KT, V = st_b["QT"], st_b["KT"], st_b["V"]
                CT = single.tile([128, KO, S], F32R, tag="CT", name=f"CT_{b}")
                mean_acc = single.tile([SP, ST, S], F32, tag="mean_acc",
                                       name=f"macc_{b}")
                st_b["CT"] = CT
                st_b["mean_acc"] = mean_acc
                for hp in range(H // 2):
                    pair = (2 * hp, 2 * hp + 1)
                    if b == 0:
                        # prefetch the NEXT pair's bias while this one computes
                        for h in (2 * hp + 2, 2 * hp + 3):
                            if h < H and h not in bT:
                                setup_bias_head(h)
                    Es = {}
                    for h in pair:
                        Es[h] = epool.tile([SP, ST, S], BF16, tag="E", name=f"E_{h}")
                    pcs = {}
                    for h in pair:
                        pcs[h] = ps_ctx.tile(
                            [DK + 1, S], F32, tag="ps_ctx", name=f"pc_{h}"
                        )
                    for kt in range(ST):
                        # columns q < kt*SP are fully causal-masked: skip them
                        off = kt * SP
                        # f32r matmuls below 256 rows fall to the 4x-slow
                        # path, so only slice the score matmul when the
                        # remaining width stays >= 256
                        soff = off if S - off >= 256 else 0
                        psts = {}
                        for h in pair:
                            pst = ps_st.tile([SP, S], F32, tag="ps_st",
                                             name=f"pst_{h}")
                            psts[h] = pst
                            nc.tensor.matmul(
                                pst[:, off:], ident_bf[:], bT[h][:, kt, off:],
                                start=True, stop=False,
                            )
                        for h in pair:
                            ko_h, base = h // 2, (h % 2) * DK
                            nc.tensor.matmul(
                                psts[h][:, soff:],
                                KT[base:base + DK, ko_h, kt * SP:(kt + 1) * SP],
                                QT[base:base + DK, ko_h, soff:],
                                start=False, stop=True,
                            )
                        for h in pair:
                            nc.scalar.activation(
                                Es[h][:, kt, off:], psts[h][:, off:], AF.Exp
                            )
                        # PV accumulation interleaved: starts after first exp
                        for h in pair:
                            nc.tensor.matmul(
                                pcs[h][:, off:],
                                V[:, kt, h, :],
                                Es[h][:, kt, off:],
                                start=(kt == 0), stop=(kt == ST - 1),
                            )

                    Ts = {pair[0]: {}, pair[1]: {}}
                    for h in pair:
                        ko_h, base = h // 2, (h % 2) * DK
                        E = Es[h]
                        pc = pcs[h]
                        # rr = 1/rowsum, replicated across partitions (K=1 mm)
                        rr = p2.tile([1, S], F32R, tag="rr", name=f"rr_{h}")
                        nc.vector.reciprocal(rr[:], pc[DK:DK + 1, :])
                        prr = ps_small.tile([SP, S], F32, tag="ps_small", name="prr")
                        nc.tensor.matmul(
                            prr[:], ones_r[:, :SP], rr[:], start=True, stop=True
                        )
                        rr_rep = p2.tile([SP, S], F32, tag="rr_rep")
                        nc.scalar.copy(rr_rep[:], prr[:])
                        rr_bf = p2.tile([SP, S], BF16, tag="rr_rep_bf")
                        nc.scalar.copy(rr_bf[:], prr[:])
                        nc.vector.tensor_tensor(
                            CT[base:base + DK, ko_h, :],
                            pc[:DK, :],
                            rr_rep[:DK, :],
                            ALU.mult,
                        )
                        for kt in range(ST):
                            off = kt * SP
                            t = tpool.tile([SP, S], BF16, tag="mtmp",
                                           name=f"T_{h}_{kt}")
                            Ts[h][kt] = t
                            nc.vector.tensor_tensor(
                                t[:, off:], E[:, kt, off:], rr_bf[:, off:],
                                ALU.mult,
                            )
                    # fold the pair into mean_acc
                    for kt in range(ST):
                        off = kt * SP
                        ta, tb = Ts[pair[0]][kt], Ts[pair[1]][kt]
                        if hp == 0:
                            eng = nc.vector if kt % 2 == 0 else nc.gpsimd
                            eng.tensor_tensor(
                                mean_acc[:, kt, off:], ta[:, off:], tb[:, off:],
                                ALU.add,
                            )
                        else:
                            psum_t = p2.tile([SP, S], BF16, tag="psum_t",
                                             name=f"ps_{hp}_{kt}")
                            nc.vector.tensor_tensor(
                                psum_t[:, off:], ta[:, off:], tb[:, off:], ALU.add
                            )
                            eng = nc.vector if kt % 2 == 0 else nc.gpsimd
                            eng.tensor_tensor(
                                mean_acc[:, kt, off:], mean_acc[:, kt, off:],
                                psum_t[:, off:], ALU.add,
                            )

            def tail_phase(b):
                st_b = state[b]
                CT, mean_acc = st_b["CT"], st_b["mean_acc"]
                # attn-mean: transpose back to [q, k]; 1/H rides the copy
                for qt in range(ST):
                    # blocks with kt > qt are entirely in the causal-masked
                    # region (k > q): the output there is exactly zero
                    w = (qt + 1) * SP
                    ptr = ps_small.tile([SP, S], F32, tag="ps_small", name="ptr_m")
                    for kt in range(qt + 1):
                        nc.tensor.transpose(
                            ptr[:, kt * SP:(kt + 1) * SP],
                            mean_acc[:, kt, qt * SP:(qt + 1) * SP],
                            ident[:SP, :SP],
                        )
                    msb_full = opool.tile([SP, D], F32, tag="ob", name="msb")
                    msb = msb_full[:, :S]
                    if w < S:
                        nc.gpsimd.memset(msb[:, w:], 0.0)
                    nc.scalar.mul(msb[:, :w], ptr[:, :w], 0.125)
                    nc.sync.dma_start(
                        amean[b, qt * SP:(qt + 1) * SP, :], msb[:]
                    )

                # output projection: out[q, :] = ctx @ wo + bo
                for qt in range(ST):
                    po = ps_big.tile([128, 512], F32, tag="ps_big", name="po")
                    for ko in range(KO):
                        nc.tensor.matmul(
                            po[:SP, :],
                            CT[:, ko, qt * SP:(qt + 1) * SP],
                            w_r["wo"][:, ko, :],
                            start=(ko == 0), stop=(ko == KO - 1),
                        )
                    osb = opool.tile([SP, D], F32, tag="ob", name="osb")
                    nc.vector.tensor_tensor(osb[:], po[:SP, :], bo_rep[:], ALU.add)
                    nc.sync.dma_start(out[b, qt * SP:(qt + 1) * SP, :], osb[:])

            for b in range(BPC):
                proj_phase(b)
                if b == 0:
                    setup_bias_head(0)
                    setup_bias_head(1)
                else:
                    tail_phase(b - 1)
                attn_phase(b)
            tail_phase(BPC - 1)

    nc.finalize()
    return nc


_NC_CACHE = None


def _get_nc():
    global _NC_CACHE
    if _NC_CACHE is None:
        _NC_CACHE = build()
    return _NC_CACHE


def run(inputs: dict, trace: bool = False):
    nc = _get_nc()
    shared = {k: np.ascontiguousarray(np.asarray(v, dtype=np.float32))
              for k, v in inputs.items() if k != "x"}
    xfull = np.ascontiguousarray(np.asarray(inputs["x"], dtype=np.float32))
    in_maps = [
        {"x": xfull[c * BPC:(c + 1) * BPC], **shared} for c in range(N_CORES)
    ]
    res = run_bass_kernel_spmd(
        nc, in_maps, core_ids=list(range(N_CORES)), trace=trace
    )
    out = np.concatenate([r["out"] for r in res.results], axis=0)
    amean = np.concatenate([r["amean"] for r in res.results], axis=0)
    return (out, amean), res


def kernel(**inputs) -> tuple[np.ndarray, np.ndarray]:
    (out, amean), _ = run(inputs, trace=False)
    return out, amean


# revision 84
# speedup vs baseline: 1.0980x; 1.0930x over previous
"""Causal MHA with relative bias + attention-mean output, on 8 trn2 cores.

Reference computation (per batch b):
  q/k/v = x @ w* + b*           -> [S, D] split into H=8 heads of dk=64
  scores = q k^T / 8 + rel_bias[:, :S, :S], causal masked
  attn = softmax(scores); ctx = attn @ v; out = ctx @ wo + bo
  second output: attn.mean(over heads)  -> [B, S, S]

Sharding: data-parallel over batch, 4 batches per core, weights/bias/rel_bias
replicated. One SPMD NEFF runs on cores 0-7.

Per-core dataflow (S=480 = 4 tiles of 120, D=512 = 4 k-tiles of 128):
  - x[b] -> PE-transpose -> xT [d, s] (f32r)
  - Q^T = wq^T @ xT (heads on partitions, scaled by 1/8), K^T likewise
  - V natural [s, d] with a ones column appended per head (row 64 of the PV
    matmul output then carries the softmax denominators for free)
  - scores are built TRANSPOSED (st[k, q]): rel_bias+causal mask (transposed
    once at setup, bf16) is injected into PSUM with an identity matmul, then
    the K^T/Q^T matmul accumulates on top; exp on the scalar engine
  - ctx^T = matmul(V_aug, E); per-q 1/rowsum (replicated across partitions
    with a K=1 ones matmul) folded into the PSUM->SBUF copy;
    out = matmul(ctx^T, wo) + bo
  - attn-mean accumulated unnormalized-by-1/H in [k, q] layout (E * rr_rep),
    PE-transposed back to natural [q, k] at the end; the 1/8 rides the final
    PSUM->SBUF copy.
"""

import numpy as np

import concourse.bass as bass
import concourse.mybir as mybir
import concourse.tile as tile
from concourse import bacc
from concourse.bass_utils import run_bass_kernel_spmd

F32 = mybir.dt.float32
F32R = mybir.dt.float32r
BF16 = mybir.dt.bfloat16
AF = mybir.ActivationFunctionType
ALU = mybir.AluOpType

B = 32
S = 480
D = 512
H = 8
DK = 64
MAX_LEN = 500
N_CORES = 8
BPC = B // N_CORES  # batches per core
ST = 4  # s tiles of 120
SP = S // ST  # 120
KO = D // 128  # 4
MASK_VAL = -1e9


def build():
    nc = bacc.Bacc("TRN2", num_devices=N_CORES)

    x = nc.dram_tensor("x", [BPC, S, D], F32, kind="ExternalInput")
    wq = nc.dram_tensor("wq", [D, D], F32, kind="ExternalInput")
    wk = nc.dram_tensor("wk", [D, D], F32, kind="ExternalInput")
    wv = nc.dram_tensor("wv", [D, D], F32, kind="ExternalInput")
    wo = nc.dram_tensor("wo", [D, D], F32, kind="ExternalInput")
    bq = nc.dram_tensor("bq", [D], F32, kind="ExternalInput")
    bk = nc.dram_tensor("bk", [D], F32, kind="ExternalInput")
    bv = nc.dram_tensor("bv", [D], F32, kind="ExternalInput")
    bo = nc.dram_tensor("bo", [D], F32, kind="ExternalInput")
    rel_bias = nc.dram_tensor("rel_bias", [H, MAX_LEN, MAX_LEN], F32, kind="ExternalInput")

    out = nc.dram_tensor("out", [BPC, S, D], F32, kind="ExternalOutput")
    amean = nc.dram_tensor("amean", [BPC, S, S], F32, kind="ExternalOutput")

    with nc.allow_low_precision("f32r pipeline is deliberate"), tile.TileContext(nc) as tc:
        with (
            tc.tile_pool(name="const", bufs=1) as const,
            tc.tile_pool(name="stage", bufs=2) as stage,
            tc.tile_pool(name="xstage", bufs=1) as xstage,
            tc.tile_pool(name="p2", bufs=4) as p2,
            tc.tile_pool(name="tpool", bufs=8) as tpool,
            tc.tile_pool(name="perb", bufs=2) as perb,
            tc.tile_pool(name="single", bufs=1) as single,
            tc.tile_pool(name="epool", bufs=4) as epool,
            tc.tile_pool(name="opool", bufs=3) as opool,
            tc.tile_pool(name="ps_big", bufs=2, space="PSUM") as ps_big,
            tc.tile_pool(name="ps_st", bufs=3, space="PSUM") as ps_st,
            tc.tile_pool(name="ps_ctx", bufs=2, space="PSUM") as ps_ctx,
            tc.tile_pool(name="ps_small", bufs=1, space="PSUM") as ps_small,
        ):
            # ---------------- setup: identities / ones ----------------
            ident = const.tile([128, 128], F32)
            nc.gpsimd.memset(ident[:], 0.0)
            nc.gpsimd.affine_select(
                out=ident[:], in_=ident[:], compare_op=ALU.not_equal, fill=1.0,
                base=0, pattern=[[-1, 128]], channel_multiplier=1,
            )
            ident_bf = const.tile([SP, SP], BF16)
            nc.gpsimd.memset(ident_bf[:], 0.0)
            nc.gpsimd.affine_select(
                out=ident_bf[:], in_=ident_bf[:], compare_op=ALU.not_equal, fill=1.0,
                base=0, pattern=[[-1, SP]], channel_multiplier=1,
            )
            ones_f = const.tile([1, 128], F32, tag="ones_f")
            nc.vector.memset(ones_f[:], 1.0)
            ones_r = const.tile([1, 128], F32R, tag="ones_r")
            nc.vector.tensor_copy(ones_r[:], ones_f[:])
            ones_col = const.tile([SP, 1], F32, tag="ones_col")
            nc.vector.memset(ones_col[:], 1.0)

            # ---------------- setup: weights -> f32r ----------------
            # [128, ko, 512]: partition = d_in % 128, ko = d_in // 128
            w_r = {}
            for i, (name, w) in enumerate(
                (("wq", wq), ("wk", wk), ("wv", wv), ("wo", wo))
            ):
                st_t = stage.tile([128, KO, D], F32, tag="stage")
                dma_eng = (nc.scalar, nc.gpsimd, nc.scalar, nc.gpsimd)[i]
                dma_eng.dma_start(st_t[:], w.rearrange("(ko p) n -> p ko n", p=128))
                wr = const.tile([128, KO, D], F32R, tag=f"w_{name}")
                nc.vector.tensor_copy(wr[:], st_t[:])
                w_r[name] = wr

            # ---------------- setup: bias vectors ----------------
            # partition layout [128, ko] for per-partition ACT bias (Q^T/K^T)
            bq_p = const.tile([128, KO], F32, tag="bq_p")
            bk_p = const.tile([128, KO], F32, tag="bk_p")
            nc.sync.dma_start(bq_p[:], bq.rearrange("(ko p) -> p ko", p=128))
            nc.sync.dma_start(bk_p[:], bk.rearrange("(ko p) -> p ko", p=128))
            bq8_p = const.tile([128, KO], F32, tag="bq8_p")
            nc.vector.tensor_scalar_mul(bq8_p[:], bq_p[:], 0.125)
            # bv/bo replicated across partitions via K=1 ones matmul
            bv_f = const.tile([1, D], F32, tag="bv_f")
            bo_f = const.tile([1, D], F32, tag="bo_f")
            nc.sync.dma_start(bv_f[:], bv[None, :])
            nc.sync.dma_start(bo_f[:], bo[None, :])
            bv_rep = const.tile([SP, H, DK], F32, tag="bv_rep")
            bo_rep = const.tile([SP, D], F32, tag="bo_rep")
            for vec, rep in ((bv_f, bv_rep), (bo_f, bo_rep)):
                prep = ps_big.tile([128, 512], F32, tag="ps_big")
                nc.tensor.matmul(
                    prep[:SP, :], ones_f[:, :SP], vec[:], start=True, stop=True
                )
                if rep is bv_rep:
                    nc.vector.tensor_copy(
                        rep[:], prep[:SP, :].rearrange("p (h d) -> p h d", h=H)
                    )
                else:
                    nc.vector.tensor_copy(rep[:], prep[:SP, :])

            # ---------------- transposed masked rel_bias (lazy, emitted
            # just-in-time inside batch 0 so it interleaves with compute) ----
            bT = {}

            def setup_bias_head(h):
                nat_full = stage.tile([128, KO, D], F32, tag="stage", name=f"nat_{h}")
                nat = nat_full[:SP, :, :S]
                nc.scalar.dma_start(
                    nat[:], rel_bias[h, :S, :S].rearrange("(qt p) k -> p qt k", p=SP)
                )
                # causal mask: only the diagonal blocks are partially
                # masked; blocks with k-tile > q-tile are never read (the
                # bias matmuls are sliced to q >= kt*SP)
                for qt in range(ST):
                    nc.gpsimd.affine_select(
                        out=nat[:, qt, qt * SP:(qt + 1) * SP],
                        in_=nat[:, qt, qt * SP:(qt + 1) * SP],
                        compare_op=ALU.is_ge, fill=MASK_VAL,
                        base=0, pattern=[[-1, SP]], channel_multiplier=1,
                    )
                bt_h = const.tile([SP, ST, S], BF16, tag=f"bT_{h}", name=f"bT_{h}")
                for kt in range(ST):
                    ptr = ps_big.tile([128, 512], F32, tag="ps_big", name="ptr_b")
                    for qt in range(kt, ST):
                        nc.tensor.transpose(
                            ptr[:SP, qt * SP:(qt + 1) * SP],
                            nat[:, qt, kt * SP:(kt + 1) * SP],
                            ident[:SP, :SP],
                        )
                    if kt % 2 == 0:
                        nc.vector.tensor_copy(
                            bt_h[:, kt, kt * SP:], ptr[:SP, kt * SP:S]
                        )
                    else:
                        nc.scalar.copy(
                            bt_h[:, kt, kt * SP:], ptr[:SP, kt * SP:S]
                        )
                bT[h] = bt_h

            # ---------------- per batch (software-pipelined) ----------------
            # emission order: proj(b) -> tail(b-1) -> attention(b); the tail
            # of a batch (out-proj + mean transposes) overlaps the next
            # batch's projection phase on otherwise-idle engines.
            state = {}

            def proj_phase(b):
                xs_full = xstage.tile([128, KO, D], F32, tag="xstage",
                                      name=f"xs_{b}")
                xs = xs_full[:SP]
                nc.sync.dma_start(
                    xs[:], x[b].rearrange("(st p) d -> p st d", p=SP)
                )

                # xT [128, ko, 480] f32r via PE transposes
                xT = single.tile([128, KO, S], F32R, tag="xT", name=f"xT_{b}")
                for ko in range(KO):
                    ptr = ps_big.tile([128, 512], F32, tag="ps_big", name="ptr_x")
                    for st_i in range(ST):
                        nc.tensor.transpose(
                            ptr[:, st_i * SP:(st_i + 1) * SP],
                            xs[:, st_i, ko * 128:(ko + 1) * 128],
                            ident[:SP, :SP],
                        )
                    nc.vector.tensor_copy(xT[:, ko, :], ptr[:, :S])

                # Q^T (scaled 1/8) and K^T: [128, ko, 480] f32r
                QT = perb.tile([128, KO, S], F32R, tag="QT", name=f"QT_{b}")
                KT = perb.tile([128, KO, S], F32R, tag="KT", name=f"KT_{b}")
                for m in range(KO):
                    pq = ps_big.tile([128, 512], F32, tag="ps_big", name="pq")
                    for ko in range(KO):
                        nc.tensor.matmul(
                            pq[:, :S],
                            w_r["wq"][:, ko, m * 128:(m + 1) * 128],
                            xT[:, ko, :],
                            start=(ko == 0), stop=(ko == KO - 1),
                        )
                    nc.scalar.activation(
                        QT[:, m, :], pq[:, :S], AF.Identity,
                        bias=bq8_p[:, m:m + 1], scale=0.125,
                    )
                    pk = ps_big.tile([128, 512], F32, tag="ps_big", name="pk")
                    for ko in range(KO):
                        nc.tensor.matmul(
                            pk[:, :S],
                            w_r["wk"][:, ko, m * 128:(m + 1) * 128],
                            xT[:, ko, :],
                            start=(ko == 0), stop=(ko == KO - 1),
                        )
                    nc.scalar.activation(
                        KT[:, m, :], pk[:, :S], AF.Identity,
                        bias=bk_p[:, m:m + 1], scale=1.0,
                    )

                # V natural with ones column: [120, st, H, 65] bf16
                V = perb.tile([SP, ST, H, DK + 1], BF16, tag="V", name=f"V_{b}")
                for st_i in range(ST):
                    pv = ps_big.tile([128, 512], F32, tag="ps_big", name="pv")
                    for ko in range(KO):
                        nc.tensor.matmul(
                            pv[:SP, :],
                            xT[:, ko, st_i * SP:(st_i + 1) * SP],
                            w_r["wv"][:, ko, :],
                            start=(ko == 0), stop=(ko == KO - 1),
                        )
                    nc.vector.tensor_tensor(
                        V[:, st_i, :, :DK],
                        pv[:SP, :].rearrange("p (h d) -> p h d", h=H),
                        bv_rep[:],
                        ALU.add,
                    )
                    nc.vector.tensor_copy(
                        V[:, st_i, :, DK],
                        ones_col[:, :].to_broadcast((SP, H)),
                    )
                state[b] = {"QT": QT, "KT": KT, "V": V}

            def attn_phase(b):
                st_b = state[b]
                QT, KT, V = st_b["QT"], st_b["KT"], st_b["V"]
                CT = single.tile([128, KO, S], F32R, tag="CT", name=f"CT_{b}")
                mean_acc = single.tile([SP, ST, S], F32, tag="mean_acc",
                                       name=f"macc_{b}")
                st_b["CT"] = CT
                st_b["mean_acc"] = mean_acc
                for hp in range(H // 2):
                    pair = (2 * hp, 2 * hp + 1)
                    if b == 0:
                        # prefetch the NEXT pair's bias while this one computes
                        for h in (2 * hp + 2, 2 * hp + 3):
                            if h < H and h not in bT:
                                setup_bias_head(h)
                    Es = {}
                    for h in pair:
                        Es[h] = epool.tile([SP, ST, S], BF16, tag="E", name=f"E_{h}")
                    pcs = {}
                    for h in pair:
                        pcs[h] = ps_ctx.tile(
                            [DK + 1, S], F32, tag="ps_ctx", name=f"pc_{h}"
                        )
                    for kt in range(ST):
                        # columns q < kt*SP are fully causal-masked: skip them
                        off = kt * SP
                        # f32r matmuls below 256 rows fall to the 4x-slow
                        # path, so only slice the score matmul when the
                        # remaining width stays >= 256
                        soff = off if S - off >= 256 else 0
                        psts = {}
                        for h in pair:
                            pst = ps_st.tile([SP, S], F32, tag="ps_st",
                                             name=f"pst_{h}")
                            psts[h] = pst
                            nc.tensor.matmul(
                                pst[:, off:], ident_bf[:], bT[h][:, kt, off:],
                                start=True, stop=False,
                            )
                        for h in pair:
                            ko_h, base = h // 2, (h % 2) * DK
                            nc.tensor.matmul(
                                psts[h][:, soff:],
                                KT[base:base + DK, ko_h, kt * SP:(kt + 1) * SP],
                                QT[base:base + DK, ko_h, soff:],
                                start=False, stop=True,
                            )
                        for h in pair:
                            nc.scalar.activation(
                                Es[h][:, kt, off:], psts[h][:, off:], AF.Exp
                            )
                        # PV accumulation interleaved: starts after first exp
                        for h in pair:
                            nc.tensor.matmul(
                                pcs[h][:, off:],
                                V[:, kt, h, :],
                                Es[h][:, kt, off:],
                                start=(kt == 0), stop=(kt == ST - 1),
                            )

                    Ts = {pair[0]: {}, pair[1]: {}}
                    for h in pair:
                        ko_h, base = h // 2, (h % 2) * DK
                        E = Es[h]
                        pc = pcs[h]
                        # rr = 1/rowsum, replicated across partitions (K=1 mm)
                        rr = p2.tile([1, S], F32R, tag="rr", name=f"rr_{h}")
                        nc.vector.reciprocal(rr[:], pc[DK:DK + 1, :])
                        prr = ps_small.tile([SP, S], F32, tag="ps_small", name="prr")
                        nc.tensor.matmul(
                            prr[:], ones_r[:, :SP], rr[:], start=True, stop=True
                        )
                        rr_rep = p2.tile([SP, S], F32, tag="rr_rep")
                        nc.scalar.copy(rr_rep[:], prr[:])
                        rr_bf = p2.tile([SP, S], BF16, tag="rr_rep_bf")
                        nc.scalar.copy(rr_bf[:], prr[:])
                        nc.vector.tensor_tensor(
                            CT[base:base + DK, ko_h, :],
                            pc[:DK, :],
                            rr_rep[:DK, :],
                            ALU.mult,
                        )
                        for kt in range(ST):
                            off = kt * SP
                            t = tpool.tile([SP, S], BF16, tag="mtmp",
                                           name=f"T_{h}_{kt}")
                            Ts[h][kt] = t
                            nc.vector.tensor_tensor(
                                t[:, off:], E[:, kt, off:], rr_bf[:, off:],
                                ALU.mult,
                            )
                    # fold the pair into mean_acc
                    for kt in range(ST):
                        off = kt * SP
                        ta, tb = Ts[pair[0]][kt], Ts[pair[1]][kt]
                        if hp == 0:
                            eng = nc.vector if kt % 2 == 0 else nc.gpsimd
                            eng.tensor_tensor(
                                mean_acc[:, kt, off:], ta[:, off:], tb[:, off:],
                                ALU.add,
                            )
                        else:
                            psum_t = p2.tile([SP, S], BF16, tag="psum_t",
                                             name=f"ps_{hp}_{kt}")
                            nc.vector.tensor_tensor(
                                psum_t[:, off:], ta[:, off:], tb[:, off:], ALU.add
                            )
                            eng = nc.vector if kt % 2 == 0 else nc.gpsimd
                            eng.tensor_tensor(
                                mean_acc[:, kt, off:], mean_acc[:, kt, off:],
                                psum_t[:, off:], ALU.add,
                            )

            def tail_phase(b):
                st_b = state[b]
                CT, mean_acc = st_b["CT"], st_b["mean_acc"]
                # attn-mean: transpose back to [q, k]; 1/H rides the copy
                for qt in range(ST):
                    # blocks with kt > qt are entirely in the causal-masked
                    # region (k > q): the output there is exactly zero
                    w = (qt + 1) * SP
                    ptr = ps_small.tile([SP, S], F32, tag="ps_small", name="ptr_m")
                    for kt in range(qt + 1):
                        nc.tensor.transpose(
                            ptr[:, kt * SP:(kt + 1) * SP],
                            mean_acc[:, kt, qt * SP:(qt + 1) * SP],
                            ident[:SP, :SP],
                        )
                    msb_full = opool.tile([SP, D], F32, tag="ob", name="msb")
                    msb = msb_full[:, :S]
                    if w < S:
                        nc.gpsimd.memset(msb[:, w:], 0.0)
                    nc.scalar.mul(msb[:, :w], ptr[:, :w], 0.125)
                    nc.sync.dma_start(
                        amean[b, qt * SP:(qt + 1) * SP, :], msb[:]
                    )

                # output projection: out[q, :] = ctx @ wo + bo
                for qt in range(ST):
                    po = ps_big.tile([128, 512], F32, tag="ps_big", name="po")
                    for ko in range(KO):
                        nc.tensor.matmul(
                            po[:SP, :],
                            CT[:, ko, qt * SP:(qt + 1) * SP],
                            w_r["wo"][:, ko, :],
                            start=(ko == 0), stop=(ko == KO - 1),
                        )
                    osb = opool.tile([SP, D], F32, tag="ob", name="osb")
                    nc.vector.tensor_tensor(osb[:], po[:SP, :], bo_rep[:], ALU.add)
                    nc.sync.dma_start(out[b, qt * SP:(qt + 1) * SP, :], osb[:])

            for b in range(BPC):
                proj_phase(b)
                if b == 0:
                    setup_bias_head(0)
                    setup_bias_head(1)
                else:
                    tail_phase(b - 1)
                attn_phase(b)
            tail_phase(BPC - 1)

    nc.finalize()
    return nc


_NC_CACHE = None


def _get_nc():
    global _NC_CACHE
    if _NC_CACHE is None:
        _NC_CACHE = build()
    return _NC_CACHE


def run(inputs: dict, trace: bool = False):
    nc = _get_nc()
    shared = {k: np.ascontiguousarray(np.asarray(v, dtype=np.float32))
              for k, v in inputs.items() if k != "x"}
    xfull = np.ascontiguousarray(np.asarray(inputs["x"], dtype=np.float32))
    in_maps = [
        {"x": xfull[c * BPC:(c + 1) * BPC], **shared} for c in range(N_CORES)
    ]
    res = run_bass_kernel_spmd(
        nc, in_maps, core_ids=list(range(N_CORES)), trace=trace
    )
    out = np.concatenate([r["out"] for r in res.results], axis=0)
    amean = np.concatenate([r["amean"] for r in res.results], axis=0)
    return (out, amean), res


def kernel(**inputs) -> tuple[np.ndarray, np.ndarray]:
    (out, amean), _ = run(inputs, trace=False)
    return out, amean
